# revision 19
# baseline (speedup 1.0000x reference)
"""Trainium2 Bass kernel for nn_Attention_63127429317226.

out[d] = sum_t softmax_d(W * r_star * q_t)[t, d] * q_t[t, d]
  T = 32768, D = 1024.  (The scalar bias b is softmax-invariant and drops out.)

Strategy: shard T across 8 cores (4096 rows each), t on partitions.

Host prep: q_pre = (c * q).fp16 with c = clamp(W*r_star, |c|>=TAU) — the beta
multiply is folded into input prep.  out_raw[d] = sum_t alpha*q_pre = c[d] *
out[d]; the host divides the final [1024] vector by c (exact recovery).

Per core, 32 chunks of [128 rows, 1024 d].  Two chunk styles balance the
ACT and DVE engines (the rowsum over d must ride one of them):
  'a' chunks: N=1024 ACTIVATE exp with inline accum_out rowsum  (ACT-heavy)
  'b' chunks: grouped big-N ACTIVATE exp (amortized overhead) + DVE
      fold-tree rowsum: three 2x-rate tensor_tensor halvings + a small
      1x tensor_reduce (2-D contiguous APs keep the DVE fast modes).
Then per chunk:  qn = q_pre * (1/s)   (DVE tensor_scalar, per-partition r)
  acc[:, b, :] += e_blk^T @ qn_blk    (PE, 8 block matmuls; the diagonal of
                                       each 128x128 block is the answer)
Epilogue: diag extract via eye-mask mul + segmented reduce -> [128, 8] per
core; host sums cores, reorders to [1024], divides by c.
"""

import os
import sys
from contextlib import ExitStack

import numpy as np

for _p in ("/opt/trn_rl_repo", "/root/.axon_site/_ro/trn_rl_repo"):
    if os.path.isdir(_p) and _p not in sys.path:
        sys.path.insert(0, _p)

import concourse.bacc as bacc
import concourse.tile as tile
from concourse import mybir
from concourse.bass_utils import run_bass_kernel_spmd

D = 1024
T = 32768
N_CORES = 8
P = 128
N_BLK = D // P  # 8
CHUNKS = T // N_CORES // P  # 32

F32 = mybir.dt.float32
FP16 = mybir.dt.float16

TAU = 2.0 ** -12

# Alternating schedule: 'a' pairs (inline ACT rowsum) and 'b' quads (big-N
# exp + DVE fold-tree rowsum).  12 a-chunks / 20 b-chunks balances ACT vs DVE.
GROUPS = ["a2", "b4", "b4", "a2", "b4", "a2", "b4", "a2", "b4", "a2", "a2"]
# Groups >= TAIL_START would run their qn-normalization and b-mode final
# reduces on ACT instead of DVE.  Measured slower (ACT Copy at ~1040ns/chunk
# lengthens the tail dependency chain vs DVE's 479ns tensor_scalar), so
# disabled; the plain DVE path below is the fastest measured configuration.
TAIL_START = 99
assert sum(int(g[1]) for g in GROUPS) == CHUNKS


def build_nc(t_shard: int):
    assert t_shard == CHUNKS * P
    nc = bacc.Bacc(None)
    qp = nc.dram_tensor("qp", [t_shard, D], FP16, kind="ExternalInput")
    eye = nc.dram_tensor("eye", [P, D], FP16, kind="ExternalInput")
    out = nc.dram_tensor("out", [P, N_BLK], F32, kind="ExternalOutput")

    import types as _types

    from concourse.vector_clock import ScopedClock as _ScopedClock

    def _minimal_drain(self, tick_clock, wait_clock):
        drain_inst = self.nc.sync.drain()
        wait_clock.add_sem_waits(
            drain_inst.ins, _ScopedClock({None: tick_clock.global_clock})
        )
        popped = self.nc._tile_sem_poison_stack.pop()
        assert popped is self._sem_poison

    mult = mybir.AluOpType.mult
    add = mybir.AluOpType.add
    Exp = mybir.ActivationFunctionType.Exp
    Copy = mybir.ActivationFunctionType.Copy

    with tile.TileContext(nc) as tc, ExitStack() as ctx:
        if os.environ.get("KERNEL_FASTEXIT", "1") == "1":
            tc._drain_and_barrier = _types.MethodType(_minimal_drain, tc)
        singles = ctx.enter_context(tc.tile_pool(name="singles", bufs=1))
        qpool = ctx.enter_context(tc.tile_pool(name="qpool", bufs=6))
        epool = ctx.enter_context(tc.tile_pool(name="epool", bufs=5))
        npool = ctx.enter_context(tc.tile_pool(name="npool", bufs=5))
        spool = ctx.enter_context(tc.tile_pool(name="spool", bufs=12))
        fpool = ctx.enter_context(tc.tile_pool(name="fpool", bufs=8))
        psum = ctx.enter_context(tc.tile_pool(name="psum", bufs=1, space="PSUM"))

        acc = psum.tile([P, N_BLK, 512], F32)

        chunk0 = 0
        for gi, g in enumerate(GROUPS):
            mode, cs = g[0], int(g[1])
            row0 = chunk0 * P
            qg = qpool.tile([P, cs * D], FP16, name="qg")
            # First two groups: issue the DMA from the ACT HWDGE ring — the SP
            # ring is busy with ~5us of const TENSOR_LOADs at kernel start, so
            # SP-issued DMAs don't move until ~7us in.
            dma_eng = nc.scalar if gi < 2 else nc.sync
            dma_eng.dma_start(
                out=qg,
                in_=qp[row0 : row0 + cs * P, :].rearrange(
                    "(p k) d -> p (k d)", p=P
                ),
            )
            e = epool.tile([P, cs * D], FP16, name="e")
            s = spool.tile([P, cs], F32, name="s")
            if mode == "a":
                for k in range(cs):
                    sl = slice(k * D, (k + 1) * D)
                    nc.scalar.activation(
                        e[:, sl], qg[:, sl], Exp, accum_out=s[:, k : k + 1]
                    )
            else:
                nc.scalar.activation(e, qg, Exp)
                for k in range(cs):
                    base = k * D
                    f1 = fpool.tile([P, 512], FP16, name="f1")
                    nc.vector.tensor_add(
                        f1, e[:, base : base + 512], e[:, base + 512 : base + D]
                    )
                    f2 = fpool.tile([P, 256], FP16, name="f2")
                    nc.vector.tensor_add(f2, f1[:, 0:256], f1[:, 256:512])
                    f3 = fpool.tile([P, 128], FP16, name="f3")
                    nc.vector.tensor_add(f3, f2[:, 0:128], f2[:, 128:256])
                    if gi >= TAIL_START:
                        scr = fpool.tile([P, 128], FP16, name="scr")
                        nc.scalar.activation(
                            scr, f3, Copy, accum_out=s[:, k : k + 1]
                        )
                    else:
                        nc.vector.tensor_reduce(
                            s[:, k : k + 1], f3, axis=mybir.AxisListType.X, op=add
                        )
            rf = spool.tile([P, cs], F32, name="rf")
            nc.vector.reciprocal(rf, s)
            qn = npool.tile([P, cs * D], FP16, name="qn")
            for k in range(cs):
                sl = slice(k * D, (k + 1) * D)
                if gi >= TAIL_START:
                    nc.scalar.activation(
                        qn[:, sl], qg[:, sl], Copy, scale=rf[:, k : k + 1]
                    )
                else:
                    nc.vector.tensor_scalar(
                        qn[:, sl], qg[:, sl], rf[:, k : k + 1], None, mult
                    )
                for b in range(N_BLK):
                    bb = slice(k * D + b * P, k * D + (b + 1) * P)
                    nc.tensor.matmul(
                        acc[:, b, :P],
                        e[:, bb],
                        qn[:, bb],
                        start=(chunk0 + k == 0),
                        stop=(chunk0 + k == CHUNKS - 1),
                    )
            chunk0 += cs

        # --- epilogue: extract the 8 block diagonals -> [P, N_BLK] ---
        eye_sb = singles.tile([P, N_BLK, P], FP16)
        nc.sync.dma_start(
            out=eye_sb, in_=eye[:].rearrange("p (b j) -> p b j", j=P)
        )
        masked = singles.tile([P, N_BLK, P], F32)
        dout = singles.tile([P, N_BLK], F32)
        h = N_BLK // 2
        for k in range(2):
            blks = slice(k * h, (k + 1) * h)
            nc.vector.tensor_mul(
                masked[:, blks, :], acc[:, blks, :P], eye_sb[:, blks, :]
            )
            nc.vector.tensor_reduce(
                dout[:, blks],
                masked[:, blks, :],
                axis=mybir.AxisListType.X,
                op=add,
            )
            nc.sync.dma_start(out=out[:, blks], in_=dout[:, blks])

    nc.compile()
    return nc


_NC_CACHE: dict = {}


def _get_nc(t_shard: int):
    if t_shard not in _NC_CACHE:
        _NC_CACHE[t_shard] = build_nc(t_shard)
    return _NC_CACHE[t_shard]


def _make_eye() -> np.ndarray:
    eye = np.zeros((P, D), dtype=np.float16)
    for b in range(N_BLK):
        eye[np.arange(P), b * P + np.arange(P)] = 1.0
    return eye


def _clamped_c(w: np.ndarray, r_star: np.ndarray) -> np.ndarray:
    c = (w.astype(np.float64) * r_star.astype(np.float64)).astype(np.float32)
    return np.where(np.abs(c) < TAU, np.copysign(np.float32(TAU), c), c)


def _make_in_maps(inputs) -> tuple:
    q_t = np.asarray(inputs["q_t"], dtype=np.float32)
    r_star = np.asarray(inputs["r_star"], dtype=np.float32)
    w = np.asarray(inputs["W"], dtype=np.float32)
    # inputs["b"] is a uniform pre-softmax bias: softmax(x + c) == softmax(x).
    c = _clamped_c(w, r_star)
    qp = (q_t * c[None, :]).astype(np.float16)
    t_shard = q_t.shape[0] // N_CORES
    shards = qp.reshape(N_CORES, t_shard, D)
    eye = _make_eye()
    return [{"qp": shards[i], "eye": eye} for i in range(N_CORES)], c, t_shard


def kernel(**inputs) -> np.ndarray:
    in_maps, c, t_shard = _make_in_maps(inputs)
    nc = _get_nc(t_shard)
    res = run_bass_kernel_spmd(nc, in_maps, core_ids=list(range(N_CORES)))
    parts = np.stack([res.results[i]["out"] for i in range(N_CORES)])  # [8,128,8]
    total = parts.astype(np.float64).sum(axis=0)  # [128, 8]
    out_raw = np.ascontiguousarray(total.T.reshape(-1))  # out_raw[b*128+p]
    return (out_raw / c).astype(np.float32)


# revision 22
# speedup vs baseline: 1.0741x; 1.0741x over previous
"""Trainium2 Bass kernel for nn_Attention_63127429317226.

out[d] = sum_t softmax_d(W * r_star * q_t)[t, d] * q_t[t, d]
  T = 32768, D = 1024.  (The scalar bias b is softmax-invariant and drops out.)

Strategy: shard T across 8 cores (4096 rows each), t on partitions.

Host prep: q_pre = (c * q).fp16 with c = clamp(W*r_star, |c|>=TAU) — the beta
multiply is folded into input prep.  out_raw[d] = sum_t alpha*q_pre = c[d] *
out[d]; the host divides the final [1024] vector by c (exact recovery).

Per core, 32 chunks of [128 rows, 1024 d].  Two chunk styles balance the
ACT and DVE engines (the rowsum over d must ride one of them):
  'a' chunks: N=1024 ACTIVATE exp with inline accum_out rowsum  (ACT-heavy)
  'b' chunks: grouped big-N ACTIVATE exp (amortized overhead) + DVE
      fold-tree rowsum: three 2x-rate tensor_tensor halvings + a small
      1x tensor_reduce (2-D contiguous APs keep the DVE fast modes).
Then per chunk:  qn = q_pre * (1/s)   (DVE tensor_scalar, per-partition r)
  acc[:, b, :] += e_blk^T @ qn_blk    (PE, 8 block matmuls; the diagonal of
                                       each 128x128 block is the answer)
Epilogue: diag extract via eye-mask mul + segmented reduce -> [128, 8] per
core; host sums cores, reorders to [1024], divides by c.
"""

import os
import sys
from contextlib import ExitStack

import numpy as np

for _p in ("/opt/trn_rl_repo", "/root/.axon_site/_ro/trn_rl_repo"):
    if os.path.isdir(_p) and _p not in sys.path:
        sys.path.insert(0, _p)

import concourse.bacc as bacc
import concourse.tile as tile
from concourse import mybir
from concourse.bass_utils import run_bass_kernel_spmd

D = 1024
T = 32768
N_CORES = 8
P = 128
N_BLK = D // P  # 8
CHUNKS = T // N_CORES // P  # 32

F32 = mybir.dt.float32
FP16 = mybir.dt.float16

TAU = 2.0 ** -12

# Alternating schedule: 'a' pairs (inline ACT rowsum) and 'b' quads (big-N
# exp + DVE fold-tree rowsum).  12 a-chunks / 20 b-chunks balances ACT vs DVE.
GROUPS = ["a2", "b4", "a2", "b4", "a2", "b4", "a2", "b4", "a2", "b4", "a2"]
# Groups >= TAIL_START would run their qn-normalization and b-mode final
# reduces on ACT instead of DVE.  Measured slower (ACT Copy at ~1040ns/chunk
# lengthens the tail dependency chain vs DVE's 479ns tensor_scalar), so
# disabled; the plain DVE path below is the fastest measured configuration.
TAIL_START = 99
assert sum(int(g[1]) for g in GROUPS) == CHUNKS


def build_nc(t_shard: int):
    assert t_shard == CHUNKS * P
    nc = bacc.Bacc(None)
    qp = nc.dram_tensor("qp", [t_shard, D], FP16, kind="ExternalInput")
    eye = nc.dram_tensor("eye", [P, D], FP16, kind="ExternalInput")
    out = nc.dram_tensor("out", [P, N_BLK], F32, kind="ExternalOutput")

    import types as _types

    from concourse.vector_clock import ScopedClock as _ScopedClock

    def _minimal_drain(self, tick_clock, wait_clock):
        drain_inst = self.nc.sync.drain()
        wait_clock.add_sem_waits(
            drain_inst.ins, _ScopedClock({None: tick_clock.global_clock})
        )
        popped = self.nc._tile_sem_poison_stack.pop()
        assert popped is self._sem_poison

    mult = mybir.AluOpType.mult
    add = mybir.AluOpType.add
    Exp = mybir.ActivationFunctionType.Exp
    Copy = mybir.ActivationFunctionType.Copy

    with tile.TileContext(nc) as tc, ExitStack() as ctx:
        if os.environ.get("KERNEL_FASTEXIT", "1") == "1":
            tc._drain_and_barrier = _types.MethodType(_minimal_drain, tc)
        singles = ctx.enter_context(tc.tile_pool(name="singles", bufs=1))
        qpool = ctx.enter_context(tc.tile_pool(name="qpool", bufs=7))
        epool = ctx.enter_context(tc.tile_pool(name="epool", bufs=6))
        npool = ctx.enter_context(tc.tile_pool(name="npool", bufs=6))
        spool = ctx.enter_context(tc.tile_pool(name="spool", bufs=12))
        fpool = ctx.enter_context(tc.tile_pool(name="fpool", bufs=8))
        psum = ctx.enter_context(tc.tile_pool(name="psum", bufs=1, space="PSUM"))

        acc = psum.tile([P, N_BLK, 512], F32)

        chunk0 = 0
        for gi, g in enumerate(GROUPS):
            mode, cs = g[0], int(g[1])
            row0 = chunk0 * P
            qg = qpool.tile([P, cs * D], FP16, name="qg")
            nc.sync.dma_start(
                out=qg,
                in_=qp[row0 : row0 + cs * P, :].rearrange(
                    "(p k) d -> p (k d)", p=P
                ),
            )
            e = epool.tile([P, cs * D], FP16, name="e")
            s = spool.tile([P, cs], F32, name="s")
            if mode == "a":
                for k in range(cs):
                    sl = slice(k * D, (k + 1) * D)
                    nc.scalar.activation(
                        e[:, sl], qg[:, sl], Exp, accum_out=s[:, k : k + 1]
                    )
            else:
                nc.scalar.activation(e, qg, Exp)
                for k in range(cs):
                    base = k * D
                    f1 = fpool.tile([P, 512], FP16, name="f1")
                    nc.vector.tensor_add(
                        f1, e[:, base : base + 512], e[:, base + 512 : base + D]
                    )
                    f2 = fpool.tile([P, 256], FP16, name="f2")
                    nc.vector.tensor_add(f2, f1[:, 0:256], f1[:, 256:512])
                    f3 = fpool.tile([P, 128], FP16, name="f3")
                    nc.vector.tensor_add(f3, f2[:, 0:128], f2[:, 128:256])
                    if gi >= TAIL_START:
                        scr = fpool.tile([P, 128], FP16, name="scr")
                        nc.scalar.activation(
                            scr, f3, Copy, accum_out=s[:, k : k + 1]
                        )
                    else:
                        nc.vector.tensor_reduce(
                            s[:, k : k + 1], f3, axis=mybir.AxisListType.X, op=add
                        )
            rf = spool.tile([P, cs], F32, name="rf")
            nc.vector.reciprocal(rf, s)
            qn = npool.tile([P, cs * D], FP16, name="qn")
            for k in range(cs):
                sl = slice(k * D, (k + 1) * D)
                if gi >= TAIL_START:
                    nc.scalar.activation(
                        qn[:, sl], qg[:, sl], Copy, scale=rf[:, k : k + 1]
                    )
                else:
                    nc.vector.tensor_scalar(
                        qn[:, sl], qg[:, sl], rf[:, k : k + 1], None, mult
                    )
                for b in range(N_BLK):
                    bb = slice(k * D + b * P, k * D + (b + 1) * P)
                    nc.tensor.matmul(
                        acc[:, b, :P],
                        e[:, bb],
                        qn[:, bb],
                        start=(chunk0 + k == 0),
                        stop=(chunk0 + k == CHUNKS - 1),
                    )
            chunk0 += cs

        # --- epilogue: extract the 8 block diagonals -> [P, N_BLK] ---
        eye_sb = singles.tile([P, N_BLK, P], FP16)
        nc.sync.dma_start(
            out=eye_sb, in_=eye[:].rearrange("p (b j) -> p b j", j=P)
        )
        masked = singles.tile([P, N_BLK, P], F32)
        dout = singles.tile([P, N_BLK], F32)
        h = N_BLK // 2
        for k in range(2):
            blks = slice(k * h, (k + 1) * h)
            nc.vector.tensor_mul(
                masked[:, blks, :], acc[:, blks, :P], eye_sb[:, blks, :]
            )
            nc.vector.tensor_reduce(
                dout[:, blks],
                masked[:, blks, :],
                axis=mybir.AxisListType.X,
                op=add,
            )
            nc.sync.dma_start(out=out[:, blks], in_=dout[:, blks])

    nc.compile()
    return nc


_NC_CACHE: dict = {}


def _get_nc(t_shard: int):
    if t_shard not in _NC_CACHE:
        _NC_CACHE[t_shard] = build_nc(t_shard)
    return _NC_CACHE[t_shard]


def _make_eye() -> np.ndarray:
    eye = np.zeros((P, D), dtype=np.float16)
    for b in range(N_BLK):
        eye[np.arange(P), b * P + np.arange(P)] = 1.0
    return eye


def _clamped_c(w: np.ndarray, r_star: np.ndarray) -> np.ndarray:
    c = (w.astype(np.float64) * r_star.astype(np.float64)).astype(np.float32)
    return np.where(np.abs(c) < TAU, np.copysign(np.float32(TAU), c), c)


def _make_in_maps(inputs) -> tuple:
    q_t = np.asarray(inputs["q_t"], dtype=np.float32)
    r_star = np.asarray(inputs["r_star"], dtype=np.float32)
    w = np.asarray(inputs["W"], dtype=np.float32)
    # inputs["b"] is a uniform pre-softmax bias: softmax(x + c) == softmax(x).
    c = _clamped_c(w, r_star)
    qp = (q_t * c[None, :]).astype(np.float16)
    t_shard = q_t.shape[0] // N_CORES
    shards = qp.reshape(N_CORES, t_shard, D)
    eye = _make_eye()
    return [{"qp": shards[i], "eye": eye} for i in range(N_CORES)], c, t_shard


def kernel(**inputs) -> np.ndarray:
    in_maps, c, t_shard = _make_in_maps(inputs)
    nc = _get_nc(t_shard)
    res = run_bass_kernel_spmd(nc, in_maps, core_ids=list(range(N_CORES)))
    parts = np.stack([res.results[i]["out"] for i in range(N_CORES)])  # [8,128,8]
    total = parts.astype(np.float64).sum(axis=0)  # [128, 8]
    out_raw = np.ascontiguousarray(total.T.reshape(-1))  # out_raw[b*128+p]
    return (out_raw / c).astype(np.float32)
